# revision 15
# baseline (speedup 1.0000x reference)
"""DynamiConv Trainium2 kernel.

out = gate * conv3x3(x, weight) + bias,  gate = conv3x3(sigmoid(x), dweight)

Strategy (per core, data-parallel over batch B=8 across 8 cores):
 - x96 [96, H, W+2] fp16 in SBUF: partition (ki*32+c) holds x[c, h+ki-1, w].
   Center block (ki=1) comes from one HWDGE DMA of host-precast fp16 x; the
   ki=0 / ki=2 blocks are built by SBUF->SBUF HWDGE DMAs (keeps DVE free).
   Input DMA runs one 32-row group ahead so the ki=2 copy of group g (which
   needs the first row of group g+1) has its data in flight.  Guard columns
   (w=-1, w=256) and boundary rows are zeroed => conv zero padding.
 - s96 = sigmoid(x96) on ACT in 32-row chunks (prep_fuse strips); guards
   become 0.5, matching the reference which sigmoids the zero-padded im2col.
 - Both convs are K=96 matmuls with the 3 horizontal taps (kj) accumulated in
   PSUM via rhs access-pattern shifts. 4 column-group lanes (tile_position)
   process 4 row-pairs concurrently; kj-major ("phase") issue order lets the
   4 lanes' matmuls overlap on distinct 32-col PE groups.
 - Output: DVE evicts gate PSUM->fp16 SBUF, DVE tensor_mul with psumY, then
   GPSIMD adds the per-partition bias (SBUF only), DMA out fp32.
"""

import os
import sys
import numpy as np
from contextlib import ExitStack

sys.path.insert(0, "/opt/trn_rl_repo")

C, O, KK = 32, 32, 3
W = 256
WG = W + 2          # guarded width (w = -1 .. 256)
TROWS = 8           # output rows per strip
LANES = 4           # column-group lanes; TROWS == 2 * LANES
NCORES = 8

_CACHE = {}


def _build_bass(H, reps=1, dataprep="host16", copies_via="dma", mm_order="phase",
                prep_fuse=4, evict_engine="vector", bias_engine="gpsimd",
                no_gate=False, skip_out=False, skip_copies=False, out_batch=4,
                skip_sigmoid=False, skip_muls=False, out_dma="gpsimd"):
    import concourse.tile as tile
    from concourse import bacc, mybir

    f32 = mybir.dt.float32
    f16 = mybir.dt.float16

    nstrips = H // TROWS
    GSTR = prep_fuse                  # strips per prep/dma group
    ngroups = nstrips // GSTR
    GR = GSTR * TROWS                 # rows per group
    if nstrips % out_batch != 0:
        out_batch = 1
    nc = bacc.Bacc("TRN2")

    x_dt = f16 if dataprep == "host16" else f32
    x_d = nc.dram_tensor("x", (C, H, W), x_dt, kind="ExternalInput")
    lm_d = nc.dram_tensor("lm", (KK, 96, O), f16, kind="ExternalInput")
    lg_d = nc.dram_tensor("lg", (KK, 96, O), f16, kind="ExternalInput")
    b_d = nc.dram_tensor("bias128", (128, 1), f32, kind="ExternalInput")
    out_d = nc.dram_tensor("out", (O, H, W), f32, kind="ExternalOutput")

    with tile.TileContext(nc) as tc, ExitStack() as ctx:
        singles = ctx.enter_context(tc.tile_pool(name="singles", bufs=1))
        s_pool = ctx.enter_context(tc.tile_pool(name="s_pool", bufs=2))
        o_pool = ctx.enter_context(tc.tile_pool(name="o_pool", bufs=3))
        g_pool = ctx.enter_context(tc.tile_pool(name="g_pool", bufs=3))
        psum = ctx.enter_context(tc.tile_pool(name="psum", bufs=4, space="PSUM"))

        x96 = singles.tile([96, H, WG], f16)
        lm = singles.tile([96, KK, O], f16)
        lg = singles.tile([96, KK, O], f16)
        bias_sb = singles.tile([128, 1], f32)

        nc.sync.dma_start(out=lm, in_=lm_d[:, :, :].rearrange("k p m -> p k m"))
        nc.sync.dma_start(out=lg, in_=lg_d[:, :, :].rearrange("k p m -> p k m"))
        nc.sync.dma_start(out=bias_sb, in_=b_d[:, :])

        # zero pad guards: w guard columns, plus ki=0 row -1 and ki=2 row H
        nc.vector.memset(x96[:, :, 0:1], 0.0)
        nc.vector.memset(x96[:, :, WG - 1 : WG], 0.0)
        nc.vector.memset(x96[0:32, 0:1, :], 0.0)
        nc.vector.memset(x96[64:96, H - 1 : H, :], 0.0)

        def dma_in(g):
            r0 = g * GR
            if dataprep == "host16":
                nc.sync.dma_start(
                    out=x96[32:64, r0 : r0 + GR, 1 : 1 + W],
                    in_=x_d[:, r0 : r0 + GR, :],
                )
            elif dataprep == "swdge_cast":
                nc.gpsimd.dma_start(
                    out=x96[32:64, r0 : r0 + GR, 1 : 1 + W],
                    in_=x_d[:, r0 : r0 + GR, :],
                )
            elif dataprep == "none":
                pass

        def do_copies(g):
            # replicate center rows into the ki=0 / ki=2 blocks for group g.
            # block0[r] = center[r-1] for r in [g0, g0+GR) (skip r=0: stays 0)
            # block2[r] = center[r+1] for r in [g0, g0+GR) (skip r=H-1)
            if skip_copies:
                return
            g0 = g * GR
            a_lo = max(g0, 1)
            b_hi = min(g0 + GR, H - 1)
            if copies_via == "dma":
                # both on the sync (SP) queue: their only dependencies are
                # earlier input DMAs on the same FIFO, so they never stall
                # another engine's sequencer.
                nc.sync.dma_start(
                    out=x96[0:32, a_lo : g0 + GR, :],
                    in_=x96[32:64, a_lo - 1 : g0 + GR - 1, :],
                )
                nc.sync.dma_start(
                    out=x96[64:96, g0:b_hi, :],
                    in_=x96[32:64, g0 + 1 : b_hi + 1, :],
                )
            else:
                nc.vector.tensor_copy(
                    out=x96[0:32, a_lo : g0 + GR, :],
                    in_=x96[32:64, a_lo - 1 : g0 + GR - 1, :],
                )
                nc.vector.tensor_copy(
                    out=x96[64:96, g0:b_hi, :],
                    in_=x96[32:64, g0 + 1 : b_hi + 1, :],
                )

        def do_sigmoid(g):
            r0 = g * GR
            s_buf = s_pool.tile([96, GR, WG], f16)
            if not skip_sigmoid:
                nc.scalar.activation(
                    out=s_buf,
                    in_=x96[:, r0 : r0 + GR, :],
                    func=mybir.ActivationFunctionType.Sigmoid,
                )
            do_sigmoid.s_buf = s_buf

        def do_strip(t):
            r0 = t * TROWS
            if t % GSTR == 0:
                do_sigmoid(t // GSTR)
            s_buf = do_sigmoid.s_buf
            s_off = (t % GSTR) * TROWS

            psumY = psum.tile([128, 2 * W], mybir.dt.float32)
            psumG = psum.tile([128, 2 * W], mybir.dt.float32)

            # "phase": interleave lanes per kj phase so the 4 column-group
            # lanes' matmuls overlap (HW-validated; CoreSim-incompatible).
            def emit(psum_t, lhsT_t, rhs_fn):
                if mm_order == "lane":
                    order = [(j, kj) for j in range(LANES) for kj in range(KK)]
                else:
                    order = [(j, kj) for kj in range(KK) for j in range(LANES)]
                for j, kj in order:
                    nc.tensor.matmul(
                        psum_t[32 * j : 32 * j + 32, :],
                        lhsT=lhsT_t[:, kj, :],
                        rhs=rhs_fn(j, kj),
                        start=(kj == 0),
                        stop=(kj == KK - 1),
                        tile_position=(0, 32 * j),
                    )

            if not skip_muls:
                emit(psumY, lm,
                     lambda j, kj: x96[:, r0 + 2 * j : r0 + 2 * j + 2, kj : kj + W])
                if not no_gate:
                    emit(psumG, lg,
                         lambda j, kj: s_buf[:, s_off + 2 * j : s_off + 2 * j + 2,
                                             kj : kj + W])

            if out_batch == 1:
                out_sb = o_pool.tile([128, 2 * W], f32)
                out_slot = out_sb
            else:
                if t % out_batch == 0:
                    do_strip.out_group = o_pool.tile([128, out_batch, 2 * W], f32)
                out_sb = do_strip.out_group
                out_slot = out_sb[:, t % out_batch, :]

            if skip_muls:
                if t == 0:
                    nc.vector.memset(out_sb, 0.0)
            elif no_gate:
                nc.vector.tensor_scalar_add(out_slot, psumY, bias_sb)
            else:
                g_sb = g_pool.tile([128, 2 * W], f16)
                if evict_engine == "vector":
                    nc.vector.tensor_copy(out=g_sb, in_=psumG)
                else:
                    nc.scalar.copy(out=g_sb, in_=psumG)
                nc.vector.tensor_mul(out=out_slot, in0=psumY, in1=g_sb)
                if bias_engine == "gpsimd":
                    nc.gpsimd.tensor_scalar_add(out_slot, out_slot, bias_sb)
                else:
                    nc.vector.tensor_scalar_add(out_slot, out_slot, bias_sb)

            if skip_out:
                if t == nstrips - 1:  # keep out_d written so it isn't DCE'd
                    nc.sync.dma_start(
                        out=out_d[:, 0:2, :], in_=out_sb[0:32, 0 : 2 * W]
                    )
            elif t % out_batch == out_batch - 1:
                g0 = t - (out_batch - 1)
                if out_dma == "gpsimd":
                    # SWDGE out-DMAs issued by gpsimd right after it computed
                    # the bias: the dependency is in-queue, so the issue never
                    # stalls another engine's sequencer (scalar-queue DMAs
                    # would block ACT's strict-FIFO on the bias semaphore).
                    ov = out_d[:, :, :].rearrange(
                        "o (tt r2) w -> o tt r2 w", r2=TROWS
                    )
                    for j in range(LANES):
                        nc.gpsimd.dma_start(
                            out=ov[:, g0 : g0 + out_batch, 2 * j : 2 * j + 2, :],
                            in_=out_sb[32 * j : 32 * j + 32, :, :],
                        )
                else:
                    ov = out_d[:, :, :].rearrange(
                        "o (tt r2) w -> o tt r2 w", r2=TROWS
                    )
                    for j in range(LANES):
                        eng = nc.sync if j % 2 == 0 else nc.scalar
                        eng.dma_start(
                            out=ov[:, g0 : g0 + out_batch, 2 * j : 2 * j + 2, :],
                            in_=out_sb[32 * j : 32 * j + 32, :, :],
                        )

        def whole_image():
            # input DMA runs two groups ahead and copies one group ahead of
            # the strips, so the sigmoid for group g never waits on the sync
            # DMA queue (the ki=2 copy of group g needs group g+1's first
            # input row).
            for i in range(nstrips + 1):
                if i < nstrips and i % GSTR == 0:
                    g = i // GSTR
                    if g == 0:
                        dma_in(0)
                        dma_in(1)
                        do_copies(0)
                    if g + 2 < ngroups:
                        dma_in(g + 2)
                    if g + 1 < ngroups:
                        do_copies(g + 1)
                if i >= 1:
                    do_strip(i - 1)

        if reps == 1:
            whole_image()
        else:
            with tc.For_i(0, reps, 1):
                whole_image()

    nc.compile()
    return nc


def _pack_inputs(x_b, weight, dweight, bias):
    # lm[kj, ki*32+c, o] = weight[o, c, ki, kj]
    lm = np.ascontiguousarray(
        weight.transpose(3, 2, 1, 0).reshape(KK, KK * C, O).astype(np.float16)
    )
    # lg[kj, ki*32+c, o] = dweight[0, c, ki, kj] for all o
    lg = np.ascontiguousarray(
        np.broadcast_to(
            dweight.transpose(3, 2, 1, 0).reshape(KK, KK * C, 1), (KK, KK * C, O)
        ).astype(np.float16)
    )
    b128 = np.tile(np.asarray(bias, dtype=np.float32), 4).reshape(128, 1)
    return {
        "x": np.ascontiguousarray(x_b, dtype=np.float16),
        "lm": lm,
        "lg": lg,
        "bias128": np.ascontiguousarray(b128),
    }


def kernel(x, weight, dweight, bias):
    from concourse import bass_utils

    x = np.asarray(x)
    weight = np.asarray(weight)
    dweight = np.asarray(dweight)
    bias = np.asarray(bias)
    B, _, H, _ = x.shape

    key = ("nc", H)
    if key not in _CACHE:
        _CACHE[key] = _build_bass(H)
    nc = _CACHE[key]

    in_maps = [_pack_inputs(x[b], weight, dweight, bias) for b in range(B)]
    # A stale terminal left by a previous session can fail the first device
    # contact (NRT_EXEC_UNIT_UNRECOVERABLE) and recover on retry — retry so a
    # one-shot caller isn't taken down by inherited device state.
    last_exc = None
    for _attempt in range(3):
        try:
            res = bass_utils.run_bass_kernel_spmd(
                nc,
                in_maps,
                core_ids=list(range(min(B, NCORES))),
                trace=bool(int(os.environ.get("DYNCONV_TRACE", "0"))),
            )
            break
        except Exception as exc:  # noqa: BLE001 - retried, re-raised below
            last_exc = exc
    else:
        raise last_exc
    out = np.stack([res.results[b]["out"] for b in range(B)], axis=0)
    kernel.last_results = res
    return out


# revision 21
# speedup vs baseline: 2.7541x; 2.7541x over previous
"""DynamiConv Trainium2 kernel.

out = gate * conv3x3(x, weight) + bias,  gate = conv3x3(sigmoid(x), dweight)

Strategy (per core, data-parallel over batch B=8 across 8 cores):
 - x96 [96, H, W+2] fp16 in SBUF: partition (ki*32+c) holds x[c, h+ki-1, w].
   Center block (ki=1) comes from plain HWDGE DMAs of host-precast fp16 x
   (halves input HBM traffic vs the old SWDGE fp32->fp16 cast and frees
   gpsimd); input runs two 8-row strips ahead and the ki=0 / ki=2 blocks are
   DVE partition-offset copies one strip ahead.  Guard columns (w=-1, w=256)
   and boundary rows are zeroed => conv zero padding.
 - s96 = sigmoid(x96 strip) on ACT; guards become 0.5, matching the
   reference which sigmoids the zero-padded im2col.
 - Both convs are K=96 matmuls with the 3 horizontal taps (kj) accumulated
   in PSUM via rhs access-pattern shifts; 4 column-group lanes
   (tile_position) cover the 4 row-pairs of a strip.
 - Output: ACT evicts gate PSUM->fp16 SBUF, DVE tensor_mul with psumY +
   tensor_scalar_add bias, batched sync/scalar HWDGE DMA out fp32.

HW A/B findings (delta-method, 12k reps; per-image times):
 - this config: ~129us. Original baseline (SWDGE-cast input, lane order):
   154-171us. The win is the host-fp16 input + deeper DMA prefetch.
 - mm_order="phase" (kj-major interleave of the 4 column-group lanes'
   accumulation groups): ~162us — WORSE than "lane". No cross-column-group
   matmul concurrency materializes for interleaved accumulation groups.
 - evict_engine="vector" + bias_engine="gpsimd": ~321us — catastrophic.
   Keep the gate evict on ACT and the bias tensor_scalar_add on DVE;
   back-to-back DVE ops stack pipeline drains and gpsimd contends DVE's
   SBUF port.
 - prep_fuse=4 (32-row sigmoid chunks): no change (~130us).
 - copies_via="dma", out_dma="gpsimd": both neutral-to-worse on the good
   config; engine copies + sync/scalar out DMA kept.
"""

import os
import sys
import numpy as np
from contextlib import ExitStack

sys.path.insert(0, "/opt/trn_rl_repo")

C, O, KK = 32, 32, 3
W = 256
WG = W + 2          # guarded width (w = -1 .. 256)
TROWS = 8           # output rows per strip
LANES = 4           # column-group lanes; TROWS == 2 * LANES
NCORES = 8

_CACHE = {}


def _build_bass(H, reps=1, dataprep="host16", copies_via="engine", mm_order="lane",
                prep_fuse=1, evict_engine="scalar", bias_engine="vector",
                no_gate=False, skip_out=False, skip_copies=False, out_batch=4,
                skip_sigmoid=False, skip_muls=False, out_dma="sync_scalar",
                skip_post=False):
    import concourse.tile as tile
    from concourse import bacc, mybir

    f32 = mybir.dt.float32
    f16 = mybir.dt.float16

    nstrips = H // TROWS
    GSTR = prep_fuse                  # strips per prep/dma group
    ngroups = nstrips // GSTR
    GR = GSTR * TROWS                 # rows per group
    if nstrips % out_batch != 0:
        out_batch = 1
    nc = bacc.Bacc("TRN2")

    x_dt = f16 if dataprep == "host16" else f32
    x_d = nc.dram_tensor("x", (C, H, W), x_dt, kind="ExternalInput")
    lm_d = nc.dram_tensor("lm", (KK, 96, O), f16, kind="ExternalInput")
    lg_d = nc.dram_tensor("lg", (KK, 96, O), f16, kind="ExternalInput")
    b_d = nc.dram_tensor("bias128", (128, 1), f32, kind="ExternalInput")
    out_d = nc.dram_tensor("out", (O, H, W), f32, kind="ExternalOutput")

    with tile.TileContext(nc) as tc, ExitStack() as ctx:
        singles = ctx.enter_context(tc.tile_pool(name="singles", bufs=1))
        s_pool = ctx.enter_context(
            tc.tile_pool(name="s_pool", bufs=3 if prep_fuse <= 2 else 2)
        )
        o_pool = ctx.enter_context(tc.tile_pool(name="o_pool", bufs=3))
        g_pool = ctx.enter_context(tc.tile_pool(name="g_pool", bufs=3))
        psum = ctx.enter_context(tc.tile_pool(name="psum", bufs=4, space="PSUM"))

        x96 = singles.tile([96, H, WG], f16)
        lm = singles.tile([96, KK, O], f16)
        lg = singles.tile([96, KK, O], f16)
        bias_sb = singles.tile([128, 1], f32)

        nc.sync.dma_start(out=lm, in_=lm_d[:, :, :].rearrange("k p m -> p k m"))
        nc.sync.dma_start(out=lg, in_=lg_d[:, :, :].rearrange("k p m -> p k m"))
        nc.sync.dma_start(out=bias_sb, in_=b_d[:, :])

        # zero pad guards: w guard columns, plus ki=0 row -1 and ki=2 row H
        nc.vector.memset(x96[:, :, 0:1], 0.0)
        nc.vector.memset(x96[:, :, WG - 1 : WG], 0.0)
        nc.vector.memset(x96[0:32, 0:1, :], 0.0)
        nc.vector.memset(x96[64:96, H - 1 : H, :], 0.0)

        def dma_in(g):
            r0 = g * GR
            if dataprep == "host16":
                nc.sync.dma_start(
                    out=x96[32:64, r0 : r0 + GR, 1 : 1 + W],
                    in_=x_d[:, r0 : r0 + GR, :],
                )
            elif dataprep == "swdge_cast":
                nc.gpsimd.dma_start(
                    out=x96[32:64, r0 : r0 + GR, 1 : 1 + W],
                    in_=x_d[:, r0 : r0 + GR, :],
                )
            elif dataprep == "none":
                pass

        def do_copies(g):
            # replicate center rows into the ki=0 / ki=2 blocks for group g.
            # block0[r] = center[r-1] for r in [g0, g0+GR) (skip r=0: stays 0)
            # block2[r] = center[r+1] for r in [g0, g0+GR) (skip r=H-1)
            if skip_copies:
                return
            g0 = g * GR
            a_lo = max(g0, 1)
            b_hi = min(g0 + GR, H - 1)
            if copies_via == "dma":
                # both on the sync (SP) queue: their only dependencies are
                # earlier input DMAs on the same FIFO, so they never stall
                # another engine's sequencer.
                nc.sync.dma_start(
                    out=x96[0:32, a_lo : g0 + GR, :],
                    in_=x96[32:64, a_lo - 1 : g0 + GR - 1, :],
                )
                nc.sync.dma_start(
                    out=x96[64:96, g0:b_hi, :],
                    in_=x96[32:64, g0 + 1 : b_hi + 1, :],
                )
            else:
                nc.vector.tensor_copy(
                    out=x96[0:32, a_lo : g0 + GR, :],
                    in_=x96[32:64, a_lo - 1 : g0 + GR - 1, :],
                )
                nc.vector.tensor_copy(
                    out=x96[64:96, g0:b_hi, :],
                    in_=x96[32:64, g0 + 1 : b_hi + 1, :],
                )

        def do_sigmoid(g):
            r0 = g * GR
            s_buf = s_pool.tile([96, GR, WG], f16)
            if skip_sigmoid:  # timing probe: keep s_buf written, ACT idle
                nc.gpsimd.memset(s_buf, 0.5)
            else:
                nc.scalar.activation(
                    out=s_buf,
                    in_=x96[:, r0 : r0 + GR, :],
                    func=mybir.ActivationFunctionType.Sigmoid,
                )
            do_sigmoid.s_buf = s_buf

        def do_strip(t):
            r0 = t * TROWS
            if t % GSTR == 0:
                do_sigmoid(t // GSTR)
            s_buf = do_sigmoid.s_buf
            s_off = (t % GSTR) * TROWS

            psumY = psum.tile([128, 2 * W], mybir.dt.float32)
            psumG = psum.tile([128, 2 * W], mybir.dt.float32)

            # "phase": interleave lanes per kj phase so the 4 column-group
            # lanes' matmuls overlap (HW-validated; CoreSim-incompatible).
            def emit(psum_t, lhsT_t, rhs_fn):
                if mm_order == "lane":
                    order = [(j, kj) for j in range(LANES) for kj in range(KK)]
                else:
                    order = [(j, kj) for kj in range(KK) for j in range(LANES)]
                for j, kj in order:
                    nc.tensor.matmul(
                        psum_t[32 * j : 32 * j + 32, :],
                        lhsT=lhsT_t[:, kj, :],
                        rhs=rhs_fn(j, kj),
                        start=(kj == 0),
                        stop=(kj == KK - 1),
                        tile_position=(0, 32 * j),
                    )

            if not skip_muls:
                emit(psumY, lm,
                     lambda j, kj: x96[:, r0 + 2 * j : r0 + 2 * j + 2, kj : kj + W])
                if not no_gate:
                    emit(psumG, lg,
                         lambda j, kj: s_buf[:, s_off + 2 * j : s_off + 2 * j + 2,
                                             kj : kj + W])

            if out_batch == 1:
                out_sb = o_pool.tile([128, 2 * W], f32)
                out_slot = out_sb
            else:
                if t % out_batch == 0:
                    do_strip.out_group = o_pool.tile([128, out_batch, 2 * W], f32)
                out_sb = do_strip.out_group
                out_slot = out_sb[:, t % out_batch, :]

            if skip_post:
                # timing probe: keep psum live with tiny DVE reads, no real
                # output stage
                nc.vector.tensor_copy(out=out_sb[:, 0, 0:1] if out_batch > 1
                                      else out_sb[:, 0:1], in_=psumY[:, 0:1])
                if not no_gate:
                    nc.vector.tensor_copy(out=out_sb[:, 0, 1:2] if out_batch > 1
                                          else out_sb[:, 1:2], in_=psumG[:, 0:1])
            elif skip_muls:
                if t == 0:
                    nc.vector.memset(out_sb, 0.0)
            elif no_gate:
                nc.vector.tensor_scalar_add(out_slot, psumY, bias_sb)
            else:
                g_sb = g_pool.tile([128, 2 * W], f16)
                if evict_engine == "vector":
                    nc.vector.tensor_copy(out=g_sb, in_=psumG)
                else:
                    nc.scalar.copy(out=g_sb, in_=psumG)
                nc.vector.tensor_mul(out=out_slot, in0=psumY, in1=g_sb)
                if bias_engine == "gpsimd":
                    nc.gpsimd.tensor_scalar_add(out_slot, out_slot, bias_sb)
                else:
                    nc.vector.tensor_scalar_add(out_slot, out_slot, bias_sb)

            if skip_out:
                if t == nstrips - 1:  # keep out_d written so it isn't DCE'd
                    nc.sync.dma_start(
                        out=out_d[:, 0:2, :], in_=out_sb[0:32, 0 : 2 * W]
                    )
            elif t % out_batch == out_batch - 1:
                g0 = t - (out_batch - 1)
                if out_dma == "gpsimd":
                    # SWDGE out-DMAs issued by gpsimd right after it computed
                    # the bias: the dependency is in-queue, so the issue never
                    # stalls another engine's sequencer (scalar-queue DMAs
                    # would block ACT's strict-FIFO on the bias semaphore).
                    ov = out_d[:, :, :].rearrange(
                        "o (tt r2) w -> o tt r2 w", r2=TROWS
                    )
                    for j in range(LANES):
                        nc.gpsimd.dma_start(
                            out=ov[:, g0 : g0 + out_batch, 2 * j : 2 * j + 2, :],
                            in_=out_sb[32 * j : 32 * j + 32, :, :],
                        )
                else:
                    ov = out_d[:, :, :].rearrange(
                        "o (tt r2) w -> o tt r2 w", r2=TROWS
                    )
                    for j in range(LANES):
                        eng = nc.sync if j % 2 == 0 else nc.scalar
                        eng.dma_start(
                            out=ov[:, g0 : g0 + out_batch, 2 * j : 2 * j + 2, :],
                            in_=out_sb[32 * j : 32 * j + 32, :, :],
                        )

        def whole_image():
            # input DMA runs two groups ahead and copies one group ahead of
            # the strips, so the sigmoid for group g never waits on the sync
            # DMA queue (the ki=2 copy of group g needs group g+1's first
            # input row).
            for i in range(nstrips + 1):
                if i < nstrips and i % GSTR == 0:
                    g = i // GSTR
                    if g == 0:
                        dma_in(0)
                        dma_in(1)
                        do_copies(0)
                    if g + 2 < ngroups:
                        dma_in(g + 2)
                    if g + 1 < ngroups:
                        do_copies(g + 1)
                if i >= 1:
                    do_strip(i - 1)

        if reps == 1:
            whole_image()
        else:
            with tc.For_i(0, reps, 1):
                whole_image()

    nc.compile()
    return nc


def _pack_inputs(x_b, weight, dweight, bias):
    # lm[kj, ki*32+c, o] = weight[o, c, ki, kj]
    lm = np.ascontiguousarray(
        weight.transpose(3, 2, 1, 0).reshape(KK, KK * C, O).astype(np.float16)
    )
    # lg[kj, ki*32+c, o] = dweight[0, c, ki, kj] for all o
    lg = np.ascontiguousarray(
        np.broadcast_to(
            dweight.transpose(3, 2, 1, 0).reshape(KK, KK * C, 1), (KK, KK * C, O)
        ).astype(np.float16)
    )
    b128 = np.tile(np.asarray(bias, dtype=np.float32), 4).reshape(128, 1)
    return {
        "x": np.ascontiguousarray(x_b, dtype=np.float16),
        "lm": lm,
        "lg": lg,
        "bias128": np.ascontiguousarray(b128),
    }


def kernel(x, weight, dweight, bias):
    from concourse import bass_utils

    x = np.asarray(x)
    weight = np.asarray(weight)
    dweight = np.asarray(dweight)
    bias = np.asarray(bias)
    B, _, H, _ = x.shape

    key = ("nc", H)
    if key not in _CACHE:
        _CACHE[key] = _build_bass(H)
    nc = _CACHE[key]

    in_maps = [_pack_inputs(x[b], weight, dweight, bias) for b in range(B)]
    # A stale terminal left by a previous session can fail the first device
    # contact (NRT_EXEC_UNIT_UNRECOVERABLE) and recover on retry — retry so a
    # one-shot caller isn't taken down by inherited device state.
    last_exc = None
    for _attempt in range(3):
        try:
            res = bass_utils.run_bass_kernel_spmd(
                nc,
                in_maps,
                core_ids=list(range(min(B, NCORES))),
                trace=bool(int(os.environ.get("DYNCONV_TRACE", "0"))),
            )
            break
        except Exception as exc:  # noqa: BLE001 - retried, re-raised below
            last_exc = exc
    else:
        raise last_exc
    out = np.stack([res.results[b]["out"] for b in range(B)], axis=0)
    kernel.last_results = res
    return out
